# revision 31
# baseline (speedup 1.0000x reference)
"""Per-channel Linear(seq->pred) over channels, 8-core channel-parallel Trainium2 kernel.

Math: y[b,p,c] = sum_s x[b,s,c] * W[c,p,s] + bias[c,p]

v4 strategy (fp8-e3m4 W stream; the kernel is HBM/DMA-bound on W):
  - W (and the bias row) are scaled by 2^8 and quantized to float8e3 (e3m4)
    on the host; x is scaled by 2^-8 and cast to bf16, so products come out
    at the right scale with no output rescale. Accumulation is fp32 in
    PSUM; output is written as bf16 and upcast on the host. Measured
    end-to-end rel err ~1.35e-2 vs the 2e-2 gate (bf16-W variant: 3.9e-3).
    This halves the dominant W stream vs bf16 (1 B/elem).
  - Channel split: 8 cores x 40 channels, and the 321st channel (index 320)
    is P-split 8 ways (90 columns per core) so no core carries a whole
    padded channel.
  - Contraction padded to 726 = 6*121 rows: row 720 is the bias (x row 720
    is 2^-8), rows 721+ are zero. K-chunks of 121 keep padding at 0.8%.
  - DMA descriptor discipline (from the v2/v3 traces): one descriptor = one
    contiguous DRAM run into one SBUF partition line; the DGE iterates the
    partition dim fastest. Oversized descriptors serialize on a single
    SBUF partition's ~14.4 GB/s write port; ~1.5-3 KB descriptors that
    cycle partitions and read sequential DRAM run fastest. So W rows are
    QUAD-interleaved: wt[quad, s, j, p] -> descriptor = 4 channels' p-rows
    = 2880 B, DRAM-sequential in s.
  - x for all 41 channel-slots (40 + shared ch320) loads once into SBUF and
    stays resident (31.5 KB/partition); x + wtail go through the gpsimd
    SWDGE queue to probe a different DMA-engine pool than the 11-engine
    HWDGE read stripe.
  - Per channel pair: Y_c[b,p] accumulates over 6 K-chunks in PSUM;
    two channels share one PSUM tile via PE column tiling (out partitions
    0:64 / 64:128). N streams as 512 + 208 (PSUM bank limit).
  - PSUM -> SBUF evacuation downcasts to bf16 (DVE for 512 cols, ACT for
    208) and DMAs out as y[c,b,p] bf16.
  - The last quad's W arrives as 6 per-chunk DMAs so its matmuls overlap
    the stream (shrinks the post-last-byte tail).
"""

import ml_dtypes
import numpy as np

import concourse.bacc as bacc
import concourse.mybir as mybir
import concourse.tile as tile
from concourse.bass_utils import run_bass_kernel_spmd

F32 = mybir.dt.float32
BF16 = mybir.dt.bfloat16
F8E3 = mybir.dt.float8e3
NP_BF16 = np.dtype(ml_dtypes.bfloat16)
NP_F8E3 = np.dtype(ml_dtypes.float8_e3m4)

B = 64          # batch
S = 720         # seq_len (contraction)
P = 720         # pred_len
C = 321         # channels
N_CORES = 8
CL = 40         # full channels per core; 8*40 = 320, channel 320 is P-split
NOCT = CL // 8  # 8-channel W interleave groups (5760 B DMA descriptors)
PTL = P // N_CORES  # tail-channel P columns per core (90)
NSLOT = CL + 1  # x slots per core (40 channels + shared channel 320)
KCH = 121       # K-chunk rows
NKCH = 6        # chunks per channel
SPAD = KCH * NKCH  # 726 padded contraction rows (720 data + bias + 5 zero)
NSPLIT = 512    # first matmul N (PSUM bank holds 512 f32)
WSC = 256.0     # W scale (x carries 1/WSC)

_CACHE: dict = {}


def _build_module():
    nc = bacc.Bacc("TRN2", target_bir_lowering=False, debug=False,
                   num_devices=N_CORES)
    wt = nc.dram_tensor("wt", [NOCT, SPAD, 8, P], F8E3, kind="ExternalInput").ap()
    wtail = nc.dram_tensor("wtail", [KCH, NKCH, PTL], F8E3, kind="ExternalInput").ap()
    xt = nc.dram_tensor("xt", [SPAD, NSLOT * B], BF16, kind="ExternalInput").ap()
    y = nc.dram_tensor("y", [CL, B, P], BF16, kind="ExternalOutput").ap()
    ytail = nc.dram_tensor("ytail", [B, PTL], BF16, kind="ExternalOutput").ap()

    with tile.TileContext(nc) as tc:
        with (
            tc.tile_pool(name="xp", bufs=1) as xp,
            tc.tile_pool(name="wp", bufs=NOCT) as wp,
            tc.tile_pool(name="pp", bufs=3, space="PSUM") as pp,
            tc.tile_pool(name="op", bufs=2) as op,
            tc.tile_pool(name="tp", bufs=1) as tp,
        ):
            # All read DMAs are configured up front, before any evac-gated
            # y-write enters an engine's config stream: a write's sem-wait
            # stalls its sequencer, which would delay later READ configs and
            # starve the (in-order) queues near the end of the stream. Every
            # W oct gets its own buffer (bufs=NOCT) so no read config ever
            # waits on a pool-rotation release. Read bytes are split evenly
            # across the two HWDGE queues (engines pipeline one descriptor
            # per queue; a lone queue only reaches ~half rate).
            xall = xp.tile([KCH, NKCH, NSLOT * B], BF16, name="xall")
            xsrc = xt.rearrange("(k s) cb -> s k cb", s=KCH)
            nc.sync.dma_start(xall[:, 0:3], xsrc[:, 0:3])
            nc.scalar.dma_start(xall[:, 3:6], xsrc[:, 3:6])
            wtl = tp.tile([KCH, NKCH, PTL], F8E3, name="wtl")
            nc.scalar.dma_start(wtl[:], wtail[:])

            wbigs = []
            for q in range(NOCT):
                wbig = wp.tile([KCH, NKCH, 8, P], F8E3, name=f"wbig{q}",
                               tag="wbig")
                wbigs.append(wbig)
                src = wt[q].rearrange("(k s) j p -> s k (j p)", s=KCH)
                if q < NOCT - 1:
                    dma_eng = nc.scalar if q % 2 == 0 else nc.sync
                    dma_eng.dma_start(
                        wbig[:].rearrange("s k j p -> s k (j p)"), src)
                else:
                    # last oct: per-chunk DMAs (alternating queues) so its
                    # matmuls overlap the loads and the tail after the last
                    # byte is one chunk's worth of work
                    for k in range(NKCH):
                        eng = nc.sync if k % 2 == 0 else nc.scalar
                        eng.dma_start(
                            wbig[:, k].rearrange("s j p -> s (j p)"),
                            src[:, k])

            # tail job: shared channel 320, this core's 90 P-columns. Runs
            # while the first oct's W is still streaming.
            pt = pp.tile([B, PTL], F32, name="pt", tag="pt", bufs=1)
            for k in range(NKCH):
                nc.tensor.matmul(pt[:, :], xall[:, k, CL * B:(CL + 1) * B],
                                 wtl[:, k, :],
                                 start=(k == 0), stop=(k == NKCH - 1))
            ot = tp.tile([B, PTL], BF16, name="ot")
            nc.vector.tensor_copy(ot[:], pt[:])
            nc.sync.dma_start(ytail[:], ot[:])

            # 5 channel octs = 20 pairs; two channels share one PSUM tile
            # via PE column tiling (output partitions 0:64 and 64:128).
            for q in range(NOCT):
                wbig = wbigs[q]
                for h in range(4):
                    c0 = 8 * q + 2 * h
                    ps = pp.tile([2 * B, P], F32, name=f"ps{c0}", tag="ps")
                    for k in range(NKCH):
                        st, sp = (k == 0), (k == NKCH - 1)
                        for half in range(2):
                            slot = c0 + half
                            lhsT = xall[:, k, slot * B:(slot + 1) * B]
                            prow = half * B
                            rhs = wbig[:, k, 2 * h + half]
                            nc.tensor.matmul(ps[prow:prow + B, 0:NSPLIT],
                                             lhsT, rhs[:, 0:NSPLIT],
                                             start=st, stop=sp)
                            nc.tensor.matmul(ps[prow:prow + B, NSPLIT:P],
                                             lhsT, rhs[:, NSPLIT:P],
                                             start=st, stop=sp)
                    out = op.tile([2 * B, P], BF16, name=f"out{c0}", tag="out")
                    nc.vector.tensor_copy(out[:, 0:NSPLIT], ps[:, 0:NSPLIT])
                    nc.vector.tensor_copy(out[:, NSPLIT:P], ps[:, NSPLIT:P])
                    out_eng = nc.sync if h % 2 == 0 else nc.scalar
                    out_eng.dma_start(
                        y[c0:c0 + 2].rearrange("c b p -> (c b) p"), out[:])

    nc.compile()
    return nc


def _get_module():
    if "nc" not in _CACHE:
        _CACHE["nc"] = _build_module()
    return _CACHE["nc"]


def _prep_inputs(x, W, b):
    # wt_all[c, s, p]: W transposed to s-major, scaled by WSC, e3m4; bias is
    # contraction row 720 (x row 720 is 1/WSC), rows 721+ zero.
    wt_all = np.zeros((C, SPAD, P), dtype=NP_F8E3)
    wt_all[:, :S, :] = (W.transpose(0, 2, 1) * WSC).astype(NP_F8E3)
    wt_all[:, S, :] = (b * WSC).astype(NP_F8E3)

    in_maps = []
    for i in range(N_CORES):
        sl = slice(i * CL, (i + 1) * CL)
        # [40, 726, 720] -> [5, 8, 726, 720] -> [5, 726, 8, 720]
        wt_core = np.ascontiguousarray(
            wt_all[sl].reshape(NOCT, 8, SPAD, P).transpose(0, 2, 1, 3))
        # shared channel 320, this core's P-slice: [121, 6, 90]
        wtail_core = np.ascontiguousarray(
            wt_all[C - 1].reshape(NKCH, KCH, P)
            .transpose(1, 0, 2)[:, :, i * PTL:(i + 1) * PTL])
        # x slots: 40 core channels + channel 320; rows s-major, scaled by
        # 1/WSC (row 720 = 1/WSC so the bias lands at scale 1)
        chs = list(range(i * CL, (i + 1) * CL)) + [C - 1]
        xrows = np.zeros((SPAD, NSLOT, B), dtype=np.float32)
        xrows[:S] = x[:, :, chs].transpose(1, 2, 0) * (1.0 / WSC)
        xrows[S] = 1.0 / WSC
        xt_core = xrows.reshape(SPAD, NSLOT * B).astype(NP_BF16)
        in_maps.append({"wt": wt_core, "wtail": wtail_core, "xt": xt_core})
    return in_maps


def _gather(results):
    yfull = np.empty((B, P, C), dtype=np.float32)
    for i in range(N_CORES):
        yc = np.asarray(results[i]["y"], dtype=np.float32)  # [40, 64, 720]
        yfull[:, :, i * CL:(i + 1) * CL] = yc.transpose(1, 2, 0)
        yt = np.asarray(results[i]["ytail"], dtype=np.float32)  # [64, 90]
        yfull[:, i * PTL:(i + 1) * PTL, C - 1] = yt
    return yfull


def run(x, W, b, **run_kwargs):
    """Full pipeline, returns (output, BassKernelResults)."""
    nc = _get_module()
    in_maps = _prep_inputs(np.asarray(x), np.asarray(W), np.asarray(b))
    res = run_bass_kernel_spmd(nc, in_maps, list(range(N_CORES)), **run_kwargs)
    return _gather(res.results), res


def kernel(x, W, b):
    out, _ = run(x, W, b)
    return out


# revision 33
# speedup vs baseline: 1.0095x; 1.0095x over previous
"""Per-channel Linear(seq->pred) over channels, 8-core channel-parallel Trainium2 kernel.

Math: y[b,p,c] = sum_s x[b,s,c] * W[c,p,s] + bias[c,p]

Final strategy (fp8-e3m4 W stream; the kernel is DMA-bound on W):
  - W (and the bias row) are scaled by 2^8 and quantized to float8e3 (e3m4)
    on the host; x is scaled by 2^-8 and cast to bf16, so products come out
    at the right scale with no output rescale. Accumulation is fp32 in
    PSUM; output is written as bf16 and upcast on the host. Measured
    end-to-end rel err 1.35e-2 vs the 2e-2 gate (bf16-W variant: 3.9e-3),
    bit-exact with the ml_dtypes host emulation. W is 1 B/elem - half the
    bf16 stream, a quarter of f32.
  - Channel split: 8 cores x 40 channels, and the 321st channel (index 320)
    is P-split 8 ways (90 columns per core) so no core carries a whole
    padded channel.
  - Contraction padded to 726 = 6*121 rows: row 720 is the bias (x row 720
    is 2^-8), rows 721+ are zero. K-chunks of 121 keep padding at 0.8%.
  - DMA descriptor discipline (measured on this part): one descriptor = one
    contiguous DRAM run into one SBUF partition line; the DGE iterates the
    partition dim fastest; descriptors dispatch in order per HWDGE queue at
    ~30 ns each, and a DMA engine overlaps one descriptor per queue (a
    single queue only reaches ~half rate - keep BOTH sync and scalar
    queues carrying balanced read bytes). Oversized (>=17 KB) descriptors
    serialize on a single partition's ~14.4 GB/s SBUF write port. So W
    rows are OCT-interleaved: wt[oct, s, j, p] -> one descriptor = 8
    channels' p-rows = 5760 B, DRAM-sequential in s. Sustained aggregate
    is capped ~195 GB/s by the chip's activity throttle (util limit 0.5).
  - x for all 41 channel-slots (40 + shared ch320) loads once into SBUF and
    stays resident (31.5 KB/partition).
  - Per channel pair: Y_c[b,p] accumulates over 6 K-chunks in PSUM;
    two channels share one PSUM tile via PE column tiling (out partitions
    0:64 / 64:128). N streams as 512 + 208 (PSUM bank limit). lhsT is the
    resident bf16 x slice; the moving operand is the fp8 W (mixed-dtype
    matmul, 1 cycle/row).
  - PSUM -> SBUF evacuation downcasts to bf16 (DVE) and DMAs out as
    y[c,b,p] bf16, writes alternating between the two queues.
  - The last oct's W arrives as 6 per-chunk DMAs so its matmuls overlap
    the stream (shrinks the post-last-byte tail).
"""

import ml_dtypes
import numpy as np

import concourse.bacc as bacc
import concourse.mybir as mybir
import concourse.tile as tile
from concourse.bass_utils import run_bass_kernel_spmd

F32 = mybir.dt.float32
BF16 = mybir.dt.bfloat16
F8E3 = mybir.dt.float8e3
NP_BF16 = np.dtype(ml_dtypes.bfloat16)
NP_F8E3 = np.dtype(ml_dtypes.float8_e3m4)

B = 64          # batch
S = 720         # seq_len (contraction)
P = 720         # pred_len
C = 321         # channels
N_CORES = 8
CL = 40         # full channels per core; 8*40 = 320, channel 320 is P-split
NOCT = CL // 8  # 8-channel W interleave groups (5760 B DMA descriptors)
PTL = P // N_CORES  # tail-channel P columns per core (90)
NSLOT = CL + 1  # x slots per core (40 channels + shared channel 320)
KCH = 121       # K-chunk rows
NKCH = 6        # chunks per channel
SPAD = KCH * NKCH  # 726 padded contraction rows (720 data + bias + 5 zero)
NSPLIT = 512    # first matmul N (PSUM bank holds 512 f32)
WSC = 256.0     # W scale (x carries 1/WSC)

_CACHE: dict = {}


def _build_module():
    nc = bacc.Bacc("TRN2", target_bir_lowering=False, debug=False,
                   num_devices=N_CORES)
    wt = nc.dram_tensor("wt", [NOCT, SPAD, 8, P], F8E3, kind="ExternalInput").ap()
    wtail = nc.dram_tensor("wtail", [KCH, NKCH, PTL], F8E3, kind="ExternalInput").ap()
    xt = nc.dram_tensor("xt", [SPAD, NSLOT * B], BF16, kind="ExternalInput").ap()
    y = nc.dram_tensor("y", [CL, B, P], BF16, kind="ExternalOutput").ap()
    ytail = nc.dram_tensor("ytail", [B, PTL], BF16, kind="ExternalOutput").ap()

    with tile.TileContext(nc) as tc:
        with (
            tc.tile_pool(name="xp", bufs=1) as xp,
            tc.tile_pool(name="wp", bufs=3) as wp,
            tc.tile_pool(name="pp", bufs=3, space="PSUM") as pp,
            tc.tile_pool(name="op", bufs=3) as op,
            tc.tile_pool(name="tp", bufs=1) as tp,
        ):
            # x (3.8 MB) rides the sync queue; W quads are split so the two
            # HWDGE queues carry near-equal read bytes and both stay busy to
            # the end of the stream (a single queue only feeds ~half the DMA
            # engines).
            xall = xp.tile([KCH, NKCH, NSLOT * B], BF16, name="xall")
            nc.sync.dma_start(xall[:], xt.rearrange("(k s) cb -> s k cb", s=KCH))
            wtl = tp.tile([KCH, NKCH, PTL], F8E3, name="wtl")
            nc.scalar.dma_start(wtl[:], wtail[:])

            # tail job first: shared channel 320, this core's 90 P-columns.
            # Runs while the first quad's W is still streaming.
            pt = pp.tile([B, PTL], F32, name="pt", tag="pt", bufs=1)
            for k in range(NKCH):
                nc.tensor.matmul(pt[:, :], xall[:, k, CL * B:(CL + 1) * B],
                                 wtl[:, k, :],
                                 start=(k == 0), stop=(k == NKCH - 1))
            ot = tp.tile([B, PTL], BF16, name="ot")
            nc.vector.tensor_copy(ot[:], pt[:])
            nc.sync.dma_start(ytail[:], ot[:])

            # 5 channel octs = 20 pairs; two channels share one PSUM tile
            # via PE column tiling (output partitions 0:64 and 64:128).
            for q in range(NOCT):
                dma_eng = nc.scalar if q % 2 == 0 else nc.sync
                wbig = wp.tile([KCH, NKCH, 8, P], F8E3, name=f"wbig{q}",
                               tag="wbig")
                src = wt[q].rearrange("(k s) j p -> s k (j p)", s=KCH)
                if q < NOCT - 1:
                    dma_eng.dma_start(
                        wbig[:].rearrange("s k j p -> s k (j p)"), src)
                else:
                    # last oct: per-chunk DMAs (alternating queues) so its
                    # matmuls overlap the loads and the final arriving chunk
                    # feeds only the last k of each pair (shrinks the tail);
                    # descriptor size stays 5760 B so dispatch cost is flat
                    for k in range(NKCH):
                        eng = nc.sync if k % 2 == 0 else nc.scalar
                        eng.dma_start(
                            wbig[:, k].rearrange("s j p -> s (j p)"),
                            src[:, k])
                for h in range(4):
                    c0 = 8 * q + 2 * h
                    ps = pp.tile([2 * B, P], F32, name=f"ps{c0}", tag="ps")
                    for k in range(NKCH):
                        st, sp = (k == 0), (k == NKCH - 1)
                        for half in range(2):
                            slot = c0 + half
                            lhsT = xall[:, k, slot * B:(slot + 1) * B]
                            prow = half * B
                            rhs = wbig[:, k, 2 * h + half]
                            nc.tensor.matmul(ps[prow:prow + B, 0:NSPLIT],
                                             lhsT, rhs[:, 0:NSPLIT],
                                             start=st, stop=sp)
                            nc.tensor.matmul(ps[prow:prow + B, NSPLIT:P],
                                             lhsT, rhs[:, NSPLIT:P],
                                             start=st, stop=sp)
                    out = op.tile([2 * B, P], BF16, name=f"out{c0}", tag="out")
                    nc.vector.tensor_copy(out[:, 0:NSPLIT], ps[:, 0:NSPLIT])
                    nc.vector.tensor_copy(out[:, NSPLIT:P], ps[:, NSPLIT:P])
                    out_eng = nc.sync if h % 2 == 0 else nc.scalar
                    out_eng.dma_start(
                        y[c0:c0 + 2].rearrange("c b p -> (c b) p"), out[:])

    nc.compile()
    return nc


def _get_module():
    if "nc" not in _CACHE:
        _CACHE["nc"] = _build_module()
    return _CACHE["nc"]


def _prep_inputs(x, W, b):
    # wt_all[c, s, p]: W transposed to s-major, scaled by WSC, e3m4; bias is
    # contraction row 720 (x row 720 is 1/WSC), rows 721+ zero.
    wt_all = np.zeros((C, SPAD, P), dtype=NP_F8E3)
    wt_all[:, :S, :] = (W.transpose(0, 2, 1) * WSC).astype(NP_F8E3)
    wt_all[:, S, :] = (b * WSC).astype(NP_F8E3)

    in_maps = []
    for i in range(N_CORES):
        sl = slice(i * CL, (i + 1) * CL)
        # [40, 726, 720] -> [5, 8, 726, 720] -> [5, 726, 8, 720]
        wt_core = np.ascontiguousarray(
            wt_all[sl].reshape(NOCT, 8, SPAD, P).transpose(0, 2, 1, 3))
        # shared channel 320, this core's P-slice: [121, 6, 90]
        wtail_core = np.ascontiguousarray(
            wt_all[C - 1].reshape(NKCH, KCH, P)
            .transpose(1, 0, 2)[:, :, i * PTL:(i + 1) * PTL])
        # x slots: 40 core channels + channel 320; rows s-major, scaled by
        # 1/WSC (row 720 = 1/WSC so the bias lands at scale 1)
        chs = list(range(i * CL, (i + 1) * CL)) + [C - 1]
        xrows = np.zeros((SPAD, NSLOT, B), dtype=np.float32)
        xrows[:S] = x[:, :, chs].transpose(1, 2, 0) * (1.0 / WSC)
        xrows[S] = 1.0 / WSC
        xt_core = xrows.reshape(SPAD, NSLOT * B).astype(NP_BF16)
        in_maps.append({"wt": wt_core, "wtail": wtail_core, "xt": xt_core})
    return in_maps


def _gather(results):
    yfull = np.empty((B, P, C), dtype=np.float32)
    for i in range(N_CORES):
        yc = np.asarray(results[i]["y"], dtype=np.float32)  # [40, 64, 720]
        yfull[:, :, i * CL:(i + 1) * CL] = yc.transpose(1, 2, 0)
        yt = np.asarray(results[i]["ytail"], dtype=np.float32)  # [64, 90]
        yfull[:, i * PTL:(i + 1) * PTL, C - 1] = yt
    return yfull


def run(x, W, b, **run_kwargs):
    """Full pipeline, returns (output, BassKernelResults)."""
    nc = _get_module()
    in_maps = _prep_inputs(np.asarray(x), np.asarray(W), np.asarray(b))
    res = run_bass_kernel_spmd(nc, in_maps, list(range(N_CORES)), **run_kwargs)
    return _gather(res.results), res


def kernel(x, W, b):
    out, _ = run(x, W, b)
    return out


# revision 37
# speedup vs baseline: 1.0420x; 1.0322x over previous
"""Per-channel Linear(seq->pred) over channels, 8-core channel-parallel Trainium2 kernel.

Math: y[b,p,c] = sum_s x[b,s,c] * W[c,p,s] + bias[c,p]

Final strategy (fp8-e3m4 W stream; the kernel is DMA-bound on W):
  - W (and the bias row) are scaled by 2^8 and quantized to float8e3 (e3m4)
    on the host; x is scaled by 2^-8 and cast to bf16, so products come out
    at the right scale with no output rescale. Accumulation is fp32 in
    PSUM; output is written as bf16 and upcast on the host. Measured
    end-to-end rel err 1.35e-2 vs the 2e-2 gate (bf16-W variant: 3.9e-3),
    bit-exact with the ml_dtypes host emulation. W is 1 B/elem - half the
    bf16 stream, a quarter of f32.
  - Channel split: 8 cores x 40 channels, and the 321st channel (index 320)
    is P-split 8 ways (90 columns per core) so no core carries a whole
    padded channel.
  - Contraction padded to 726 = 6*121 rows: row 720 is the bias (x row 720
    is 2^-8), rows 721+ are zero. K-chunks of 121 keep padding at 0.8%.
  - DMA descriptor discipline (measured on this part): one descriptor = one
    contiguous DRAM run into one SBUF partition line; the DGE iterates the
    partition dim fastest; descriptors dispatch in order per HWDGE queue at
    ~30 ns each, and a DMA engine overlaps one descriptor per queue (a
    single queue only reaches ~half rate - keep BOTH sync and scalar
    queues carrying balanced read bytes). Oversized (>=17 KB) descriptors
    serialize on a single partition's ~14.4 GB/s SBUF write port. So W
    rows are OCT-interleaved: wt[oct, s, j, p] -> one descriptor = 8
    channels' p-rows = 5760 B, DRAM-sequential in s. Sustained aggregate
    is capped ~195 GB/s by the chip's activity throttle (util limit 0.5).
  - x for all 41 channel-slots (40 + shared ch320) loads once into SBUF and
    stays resident (31.5 KB/partition).
  - Per channel pair: Y_c[b,p] accumulates over 6 K-chunks in PSUM;
    two channels share one PSUM tile via PE column tiling (out partitions
    0:64 / 64:128). N streams as 512 + 208 (PSUM bank limit). lhsT is the
    resident bf16 x slice; the moving operand is the fp8 W (mixed-dtype
    matmul, 1 cycle/row).
  - PSUM -> SBUF evacuation downcasts to bf16 (DVE) and DMAs out as
    y[c,b,p] bf16, writes alternating between the two queues.
  - The last oct's W arrives as 6 per-chunk DMAs so its matmuls overlap
    the stream (shrinks the post-last-byte tail).
"""

import ml_dtypes
import numpy as np

import concourse.bacc as bacc
import concourse.mybir as mybir
import concourse.tile as tile
from concourse.bass_utils import run_bass_kernel_spmd

F32 = mybir.dt.float32
BF16 = mybir.dt.bfloat16
F8E3 = mybir.dt.float8e3
NP_BF16 = np.dtype(ml_dtypes.bfloat16)
NP_F8E3 = np.dtype(ml_dtypes.float8_e3m4)

B = 64          # batch
S = 720         # seq_len (contraction)
P = 720         # pred_len
C = 321         # channels
N_CORES = 8
CL = 40         # full channels per core; 8*40 = 320, channel 320 is P-split
NOCT = CL // 8  # 8-channel W interleave groups (5760 B DMA descriptors)
PTL = P // N_CORES  # tail-channel P columns per core (90)
NSLOT = CL + 1  # x slots per core (40 channels + shared channel 320)
KCH = 121       # K-chunk rows
NKCH = 6        # chunks per channel
SPAD = KCH * NKCH  # 726 padded contraction rows (720 data + bias + 5 zero)
NSPLIT = 512    # first matmul N (PSUM bank holds 512 f32)
WSC = 256.0     # W scale (x carries 1/WSC)

_CACHE: dict = {}


def _build_module():
    nc = bacc.Bacc("TRN2", target_bir_lowering=False, debug=False,
                   num_devices=N_CORES)
    wt = nc.dram_tensor("wt", [NOCT, SPAD, 8, P], F8E3, kind="ExternalInput").ap()
    wtail = nc.dram_tensor("wtail", [KCH, NKCH, PTL], F8E3, kind="ExternalInput").ap()
    xt = nc.dram_tensor("xt", [SPAD, NSLOT * B], BF16, kind="ExternalInput").ap()
    y = nc.dram_tensor("y", [CL, B, P], BF16, kind="ExternalOutput").ap()
    ytail = nc.dram_tensor("ytail", [B, PTL], BF16, kind="ExternalOutput").ap()

    with tile.TileContext(nc) as tc:
        with (
            tc.tile_pool(name="xp", bufs=1) as xp,
            tc.tile_pool(name="wp", bufs=4) as wp,
            tc.tile_pool(name="pp", bufs=3, space="PSUM") as pp,
            tc.tile_pool(name="op", bufs=8) as op,
            tc.tile_pool(name="tp", bufs=1) as tp,
        ):
            # x (3.8 MB) rides the sync queue; W quads are split so the two
            # HWDGE queues carry near-equal read bytes and both stay busy to
            # the end of the stream (a single queue only feeds ~half the DMA
            # engines).
            xall = xp.tile([KCH, NKCH, NSLOT * B], BF16, name="xall")
            nc.sync.dma_start(xall[:], xt.rearrange("(k s) cb -> s k cb", s=KCH))
            wtl = tp.tile([KCH, NKCH, PTL], F8E3, name="wtl")
            nc.scalar.dma_start(wtl[:], wtail[:])

            # tail job first: shared channel 320, this core's 90 P-columns.
            # Runs while the first quad's W is still streaming.
            pt = pp.tile([B, PTL], F32, name="pt", tag="pt", bufs=1)
            for k in range(NKCH):
                nc.tensor.matmul(pt[:, :], xall[:, k, CL * B:(CL + 1) * B],
                                 wtl[:, k, :],
                                 start=(k == 0), stop=(k == NKCH - 1))
            ot = tp.tile([B, PTL], BF16, name="ot")
            nc.vector.tensor_copy(ot[:], pt[:])

            # 5 channel octs = 20 pairs; two channels share one PSUM tile
            # via PE column tiling (output partitions 0:64 and 64:128).
            # y-writes are DEFERRED by one oct: a write's DGE config stalls
            # its sequencer until the evac it depends on completes, and that
            # stall would delay every later READ config behind it in program
            # order, starving the in-order queues near the end of the
            # stream. Issued one oct late, the evac is long done and write
            # configs never stall.
            pending = [(ytail[:], ot, nc.sync)]
            for q in range(NOCT):
                dma_eng = nc.scalar if q % 2 == 0 else nc.sync
                wbig = wp.tile([KCH, NKCH, 8, P], F8E3, name=f"wbig{q}",
                               tag="wbig")
                src = wt[q].rearrange("(k s) j p -> s k (j p)", s=KCH)
                if q < NOCT - 1:
                    dma_eng.dma_start(
                        wbig[:].rearrange("s k j p -> s k (j p)"), src)
                else:
                    # last oct: per-chunk DMAs (alternating queues) so its
                    # matmuls overlap the loads and the final arriving chunk
                    # feeds only the last k of each pair (shrinks the tail);
                    # descriptor size stays 5760 B so dispatch cost is flat
                    for k in range(NKCH):
                        eng = nc.sync if k % 2 == 0 else nc.scalar
                        eng.dma_start(
                            wbig[:, k].rearrange("s j p -> s (j p)"),
                            src[:, k])
                for dst, srctile, eng in pending:
                    eng.dma_start(dst, srctile[:])
                pending = []
                for h in range(4):
                    c0 = 8 * q + 2 * h
                    ps = pp.tile([2 * B, P], F32, name=f"ps{c0}", tag="ps")
                    for k in range(NKCH):
                        st, sp = (k == 0), (k == NKCH - 1)
                        for half in range(2):
                            slot = c0 + half
                            lhsT = xall[:, k, slot * B:(slot + 1) * B]
                            prow = half * B
                            rhs = wbig[:, k, 2 * h + half]
                            nc.tensor.matmul(ps[prow:prow + B, 0:NSPLIT],
                                             lhsT, rhs[:, 0:NSPLIT],
                                             start=st, stop=sp)
                            nc.tensor.matmul(ps[prow:prow + B, NSPLIT:P],
                                             lhsT, rhs[:, NSPLIT:P],
                                             start=st, stop=sp)
                    out = op.tile([2 * B, P], BF16, name=f"out{c0}", tag="out")
                    nc.vector.tensor_copy(out[:, 0:NSPLIT], ps[:, 0:NSPLIT])
                    nc.vector.tensor_copy(out[:, NSPLIT:P], ps[:, NSPLIT:P])
                    out_eng = nc.sync if h % 2 == 0 else nc.scalar
                    pending.append(
                        (y[c0:c0 + 2].rearrange("c b p -> (c b) p"),
                         out, out_eng))
            for dst, srctile, eng in pending:
                eng.dma_start(dst, srctile[:])

    nc.compile()
    return nc


def _get_module():
    if "nc" not in _CACHE:
        _CACHE["nc"] = _build_module()
    return _CACHE["nc"]


def _prep_inputs(x, W, b):
    # wt_all[c, s, p]: W transposed to s-major, scaled by WSC, e3m4; bias is
    # contraction row 720 (x row 720 is 1/WSC), rows 721+ zero.
    wt_all = np.zeros((C, SPAD, P), dtype=NP_F8E3)
    wt_all[:, :S, :] = (W.transpose(0, 2, 1) * WSC).astype(NP_F8E3)
    wt_all[:, S, :] = (b * WSC).astype(NP_F8E3)

    in_maps = []
    for i in range(N_CORES):
        sl = slice(i * CL, (i + 1) * CL)
        # [40, 726, 720] -> [5, 8, 726, 720] -> [5, 726, 8, 720]
        wt_core = np.ascontiguousarray(
            wt_all[sl].reshape(NOCT, 8, SPAD, P).transpose(0, 2, 1, 3))
        # shared channel 320, this core's P-slice: [121, 6, 90]
        wtail_core = np.ascontiguousarray(
            wt_all[C - 1].reshape(NKCH, KCH, P)
            .transpose(1, 0, 2)[:, :, i * PTL:(i + 1) * PTL])
        # x slots: 40 core channels + channel 320; rows s-major, scaled by
        # 1/WSC (row 720 = 1/WSC so the bias lands at scale 1)
        chs = list(range(i * CL, (i + 1) * CL)) + [C - 1]
        xrows = np.zeros((SPAD, NSLOT, B), dtype=np.float32)
        xrows[:S] = x[:, :, chs].transpose(1, 2, 0) * (1.0 / WSC)
        xrows[S] = 1.0 / WSC
        xt_core = xrows.reshape(SPAD, NSLOT * B).astype(NP_BF16)
        in_maps.append({"wt": wt_core, "wtail": wtail_core, "xt": xt_core})
    return in_maps


def _gather(results):
    yfull = np.empty((B, P, C), dtype=np.float32)
    for i in range(N_CORES):
        yc = np.asarray(results[i]["y"], dtype=np.float32)  # [40, 64, 720]
        yfull[:, :, i * CL:(i + 1) * CL] = yc.transpose(1, 2, 0)
        yt = np.asarray(results[i]["ytail"], dtype=np.float32)  # [64, 90]
        yfull[:, i * PTL:(i + 1) * PTL, C - 1] = yt
    return yfull


def run(x, W, b, **run_kwargs):
    """Full pipeline, returns (output, BassKernelResults)."""
    nc = _get_module()
    in_maps = _prep_inputs(np.asarray(x), np.asarray(W), np.asarray(b))
    res = run_bass_kernel_spmd(nc, in_maps, list(range(N_CORES)), **run_kwargs)
    return _gather(res.results), res


def kernel(x, W, b):
    out, _ = run(x, W, b)
    return out


# revision 38
# speedup vs baseline: 1.1324x; 1.0868x over previous
"""Per-channel Linear(seq->pred) over channels, 8-core channel-parallel Trainium2 kernel.

Math: y[b,p,c] = sum_s x[b,s,c] * W[c,p,s] + bias[c,p]

Final strategy (fp8-e3m4 W stream; the kernel is DMA-bound on W):
  - W (and the bias row) are scaled by 2^8 and quantized to float8e3 (e3m4)
    on the host; x is scaled by 2^-8 and cast to bf16, so products come out
    at the right scale with no output rescale. Accumulation is fp32 in
    PSUM; output is written as bf16 and upcast on the host. Measured
    end-to-end rel err 1.35e-2 vs the 2e-2 gate (bf16-W variant: 3.9e-3),
    bit-exact with the ml_dtypes host emulation. W is 1 B/elem - half the
    bf16 stream, a quarter of f32.
  - Channel split: 8 cores x 40 channels, and the 321st channel (index 320)
    is P-split 8 ways (90 columns per core) so no core carries a whole
    padded channel.
  - Contraction padded to 726 = 6*121 rows: row 720 is the bias (x row 720
    is 2^-8), rows 721+ are zero. K-chunks of 121 keep padding at 0.8%.
  - DMA descriptor discipline (measured on this part): one descriptor = one
    contiguous DRAM run into one SBUF partition line; the DGE iterates the
    partition dim fastest; descriptors dispatch in order per HWDGE queue at
    ~30 ns each, and a DMA engine overlaps one descriptor per queue (a
    single queue only reaches ~half rate - keep BOTH sync and scalar
    queues carrying balanced read bytes). Oversized (>=17 KB) descriptors
    serialize on a single partition's ~14.4 GB/s SBUF write port. So W
    rows are OCT-interleaved: wt[oct, s, j, p] -> one descriptor = 8
    channels' p-rows = 5760 B, DRAM-sequential in s. Sustained aggregate
    is capped ~195 GB/s by the chip's activity throttle (util limit 0.5).
  - x for all 41 channel-slots (40 + shared ch320) loads once into SBUF and
    stays resident (31.5 KB/partition).
  - Per channel pair: Y_c[b,p] accumulates over 6 K-chunks in PSUM;
    two channels share one PSUM tile via PE column tiling (out partitions
    0:64 / 64:128). N streams as 512 + 208 (PSUM bank limit). lhsT is the
    resident bf16 x slice; the moving operand is the fp8 W (mixed-dtype
    matmul, 1 cycle/row).
  - PSUM -> SBUF evacuation downcasts to bf16 (DVE) and DMAs out as
    y[c,b,p] bf16, writes alternating between the two queues.
  - The last oct's W arrives as 6 per-chunk DMAs so its matmuls overlap
    the stream (shrinks the post-last-byte tail).
"""

import ml_dtypes
import numpy as np

import concourse.bacc as bacc
import concourse.mybir as mybir
import concourse.tile as tile
from concourse.bass_utils import run_bass_kernel_spmd

F32 = mybir.dt.float32
BF16 = mybir.dt.bfloat16
F8E3 = mybir.dt.float8e3
NP_BF16 = np.dtype(ml_dtypes.bfloat16)
NP_F8E3 = np.dtype(ml_dtypes.float8_e3m4)

B = 64          # batch
S = 720         # seq_len (contraction)
P = 720         # pred_len
C = 321         # channels
N_CORES = 8
CL = 40         # full channels per core; 8*40 = 320, channel 320 is P-split
NOCT = CL // 8  # 8-channel W interleave groups (5760 B DMA descriptors)
PTL = P // N_CORES  # tail-channel P columns per core (90)
NSLOT = CL + 1  # x slots per core (40 channels + shared channel 320)
KCH = 121       # K-chunk rows
NKCH = 6        # chunks per channel
SPAD = KCH * NKCH  # 726 padded contraction rows (720 data + bias + 5 zero)
NSPLIT = 512    # first matmul N (PSUM bank holds 512 f32)
WSC = 256.0     # W scale (x carries 1/WSC)

_CACHE: dict = {}


def _build_module():
    nc = bacc.Bacc("TRN2", target_bir_lowering=False, debug=False,
                   num_devices=N_CORES)
    wt = nc.dram_tensor("wt", [NOCT, SPAD, 8, P], F8E3, kind="ExternalInput").ap()
    wtail = nc.dram_tensor("wtail", [KCH, NKCH, PTL], F8E3, kind="ExternalInput").ap()
    xt = nc.dram_tensor("xt", [SPAD, NSLOT * B], BF16, kind="ExternalInput").ap()
    y = nc.dram_tensor("y", [CL, B, P], BF16, kind="ExternalOutput").ap()
    ytail = nc.dram_tensor("ytail", [B, PTL], BF16, kind="ExternalOutput").ap()

    with tile.TileContext(nc) as tc:
        with (
            tc.tile_pool(name="xp", bufs=1) as xp,
            tc.tile_pool(name="wp", bufs=4) as wp,
            tc.tile_pool(name="pp", bufs=3, space="PSUM") as pp,
            tc.tile_pool(name="op", bufs=8) as op,
            tc.tile_pool(name="tp", bufs=1) as tp,
        ):
            # x (3.8 MB) rides the sync queue; W quads are split so the two
            # HWDGE queues carry near-equal read bytes and both stay busy to
            # the end of the stream (a single queue only feeds ~half the DMA
            # engines).
            # x is split across the two queues so both carry ~11.4 MB of
            # reads and drain together (a lone queue runs at ~half rate)
            xall = xp.tile([KCH, NKCH, NSLOT * B], BF16, name="xall")
            xsrc = xt.rearrange("(k s) cb -> s k cb", s=KCH)
            nc.sync.dma_start(xall[:, 0:3], xsrc[:, 0:3])
            nc.scalar.dma_start(xall[:, 3:6], xsrc[:, 3:6])
            wtl = tp.tile([KCH, NKCH, PTL], F8E3, name="wtl")
            nc.scalar.dma_start(wtl[:], wtail[:])

            # tail job first: shared channel 320, this core's 90 P-columns.
            # Runs while the first quad's W is still streaming.
            pt = pp.tile([B, PTL], F32, name="pt", tag="pt", bufs=1)
            for k in range(NKCH):
                nc.tensor.matmul(pt[:, :], xall[:, k, CL * B:(CL + 1) * B],
                                 wtl[:, k, :],
                                 start=(k == 0), stop=(k == NKCH - 1))
            ot = tp.tile([B, PTL], BF16, name="ot")
            nc.vector.tensor_copy(ot[:], pt[:])

            # 5 channel octs = 20 pairs; two channels share one PSUM tile
            # via PE column tiling (output partitions 0:64 and 64:128).
            # y-writes are DEFERRED by one oct: a write's DGE config stalls
            # its sequencer until the evac it depends on completes, and that
            # stall would delay every later READ config behind it in program
            # order, starving the in-order queues near the end of the
            # stream. Issued one oct late, the evac is long done and write
            # configs never stall.
            pending = [(ytail[:], ot, nc.sync)]
            for q in range(NOCT):
                dma_eng = nc.scalar if q % 2 == 0 else nc.sync
                wbig = wp.tile([KCH, NKCH, 8, P], F8E3, name=f"wbig{q}",
                               tag="wbig")
                src = wt[q].rearrange("(k s) j p -> s k (j p)", s=KCH)
                if q < NOCT - 1:
                    dma_eng.dma_start(
                        wbig[:].rearrange("s k j p -> s k (j p)"), src)
                else:
                    # last oct: per-chunk DMAs (alternating queues) so its
                    # matmuls overlap the loads and the final arriving chunk
                    # feeds only the last k of each pair (shrinks the tail);
                    # descriptor size stays 5760 B so dispatch cost is flat
                    for k in range(NKCH):
                        eng = nc.sync if k % 2 == 0 else nc.scalar
                        eng.dma_start(
                            wbig[:, k].rearrange("s j p -> s (j p)"),
                            src[:, k])
                for dst, srctile, eng in pending:
                    eng.dma_start(dst, srctile[:])
                pending = []
                for h in range(4):
                    c0 = 8 * q + 2 * h
                    ps = pp.tile([2 * B, P], F32, name=f"ps{c0}", tag="ps")
                    for k in range(NKCH):
                        st, sp = (k == 0), (k == NKCH - 1)
                        for half in range(2):
                            slot = c0 + half
                            lhsT = xall[:, k, slot * B:(slot + 1) * B]
                            prow = half * B
                            rhs = wbig[:, k, 2 * h + half]
                            nc.tensor.matmul(ps[prow:prow + B, 0:NSPLIT],
                                             lhsT, rhs[:, 0:NSPLIT],
                                             start=st, stop=sp)
                            nc.tensor.matmul(ps[prow:prow + B, NSPLIT:P],
                                             lhsT, rhs[:, NSPLIT:P],
                                             start=st, stop=sp)
                    out = op.tile([2 * B, P], BF16, name=f"out{c0}", tag="out")
                    nc.vector.tensor_copy(out[:, 0:NSPLIT], ps[:, 0:NSPLIT])
                    nc.vector.tensor_copy(out[:, NSPLIT:P], ps[:, NSPLIT:P])
                    out_eng = nc.sync if h % 2 == 0 else nc.scalar
                    pending.append(
                        (y[c0:c0 + 2].rearrange("c b p -> (c b) p"),
                         out, out_eng))
            for dst, srctile, eng in pending:
                eng.dma_start(dst, srctile[:])

    nc.compile()
    return nc


def _get_module():
    if "nc" not in _CACHE:
        _CACHE["nc"] = _build_module()
    return _CACHE["nc"]


def _prep_inputs(x, W, b):
    # wt_all[c, s, p]: W transposed to s-major, scaled by WSC, e3m4; bias is
    # contraction row 720 (x row 720 is 1/WSC), rows 721+ zero.
    wt_all = np.zeros((C, SPAD, P), dtype=NP_F8E3)
    wt_all[:, :S, :] = (W.transpose(0, 2, 1) * WSC).astype(NP_F8E3)
    wt_all[:, S, :] = (b * WSC).astype(NP_F8E3)

    in_maps = []
    for i in range(N_CORES):
        sl = slice(i * CL, (i + 1) * CL)
        # [40, 726, 720] -> [5, 8, 726, 720] -> [5, 726, 8, 720]
        wt_core = np.ascontiguousarray(
            wt_all[sl].reshape(NOCT, 8, SPAD, P).transpose(0, 2, 1, 3))
        # shared channel 320, this core's P-slice: [121, 6, 90]
        wtail_core = np.ascontiguousarray(
            wt_all[C - 1].reshape(NKCH, KCH, P)
            .transpose(1, 0, 2)[:, :, i * PTL:(i + 1) * PTL])
        # x slots: 40 core channels + channel 320; rows s-major, scaled by
        # 1/WSC (row 720 = 1/WSC so the bias lands at scale 1)
        chs = list(range(i * CL, (i + 1) * CL)) + [C - 1]
        xrows = np.zeros((SPAD, NSLOT, B), dtype=np.float32)
        xrows[:S] = x[:, :, chs].transpose(1, 2, 0) * (1.0 / WSC)
        xrows[S] = 1.0 / WSC
        xt_core = xrows.reshape(SPAD, NSLOT * B).astype(NP_BF16)
        in_maps.append({"wt": wt_core, "wtail": wtail_core, "xt": xt_core})
    return in_maps


def _gather(results):
    yfull = np.empty((B, P, C), dtype=np.float32)
    for i in range(N_CORES):
        yc = np.asarray(results[i]["y"], dtype=np.float32)  # [40, 64, 720]
        yfull[:, :, i * CL:(i + 1) * CL] = yc.transpose(1, 2, 0)
        yt = np.asarray(results[i]["ytail"], dtype=np.float32)  # [64, 90]
        yfull[:, i * PTL:(i + 1) * PTL, C - 1] = yt
    return yfull


def run(x, W, b, **run_kwargs):
    """Full pipeline, returns (output, BassKernelResults)."""
    nc = _get_module()
    in_maps = _prep_inputs(np.asarray(x), np.asarray(W), np.asarray(b))
    res = run_bass_kernel_spmd(nc, in_maps, list(range(N_CORES)), **run_kwargs)
    return _gather(res.results), res


def kernel(x, W, b):
    out, _ = run(x, W, b)
    return out
